# revision 1
# baseline (speedup 1.0000x reference)
"""Trainium2 Bass kernel for nn_ProbAttention (sparse attention / Informer ProbSparse).

Strategy (8 NeuronCores, no collectives):
  core c -> (batch b = c//2, half h = c%2).
  Both cores of a pair compute the full attention/context for their batch;
  the big Wfin product is column-sharded across the pair (n-halves).
  Host sums the partial class scores and adds bfin.

Device pipeline per core (one batch):
  1. K^T, Q^T = W @ X^T (fp32), Q-natural / V / R^T(Wadd) in fp32r.
  2. QK = Q @ K^T (fp32) tile-by-tile; fused DVE tensor_tensor_reduce with
     host-built sample masks gives M = max_s(QK_sample) - sum_s(QK_sample)/N.
  3. gpsimd kth_largest -> threshold t = 140th largest M; select M >= t,
     compact indices with gpsimd sparse_gather (ties resolved in index order,
     matching jax.lax.top_k).
  4. One-hot matmuls gather Q_red; scores^T = K @ Q_red^T; exp on ACT;
     denominators + attn@V on PE; context built transposed via a rank-141
     scatter matmul (out_top rows + Vmean*(1-hit)) + Wadd residual + badd.
  5. Final 10-class scores: fused DVE multiply-reduce against the core's
     Wfin half, partition-reduced by a ones-matmul.

kernel(**inputs) is self-contained: host does layout prep (transposes, count
masks from index_sample, Wfin reshape) and final gather.
"""

import math
import sys

import numpy as np

sys.path.insert(0, "/opt/trn_rl_repo")

import concourse.bass as bass  # noqa: E402
import concourse.bacc as bacc  # noqa: E402
import concourse.tile as tile  # noqa: E402
from concourse import mybir  # noqa: E402
from concourse.bass_utils import run_bass_kernel_spmd  # noqa: E402

import ml_dtypes  # noqa: E402

B, N, D, NCLS, U = 4, 1024, 256, 10, 140
NEG = -30000.0
F32 = mybir.dt.float32
F32R = mybir.dt.float32r
BF16 = mybir.dt.bfloat16
F16 = mybir.dt.float16
ALU = mybir.AluOpType
ACTF = mybir.ActivationFunctionType

# kth_largest quantile: need k_adj = floor((1-q)*1023) == 138 so that the
# second output is desc[139] = the 140th largest of 1024.
KTH_Q = 1.0 - 581000000.0 / 4294967296.0


def r32(ap):
    return ap.bitcast(F32R)


def build_nc(stage=6):
    nc = bacc.Bacc("TRN2", target_bir_lowering=False, debug=False, num_devices=8)

    am_d = nc.declare_dram_parameter("amask", [N, N], BF16, isOutput=False)
    id_d = nc.declare_dram_parameter("ident_b", [128, 128], BF16, isOutput=False)
    misc_d = nc.declare_dram_parameter("misc", [128, 523], F32, isOutput=False)
    wall_d = nc.declare_dram_parameter("w_all_r", [8, 128, D], F32R, isOutput=False)
    ct_d = nc.declare_dram_parameter("ct_b", [N, N], BF16, isOutput=False)
    xb_d = nc.declare_dram_parameter("x_b", [N, D], BF16, isOutput=False)
    xw_d = nc.declare_dram_parameter("xtw2_b", [D, N + D], BF16, isOutput=False)
    wf_d = nc.declare_dram_parameter("wfin_r", [NCLS, 128, 2 * 512], BF16, isOutput=False)
    iwrap1_d = nc.declare_dram_parameter("iwrap1", [16, 64], F32, isOutput=False)
    xtr_d = nc.declare_dram_parameter("xt_r", [D, N], F32R, isOutput=False)
    onesr_d = nc.declare_dram_parameter("ones_r", [128, 1], F32R, isOutput=False)
    out_d = nc.declare_dram_parameter("out20", [1, NCLS], F32, isOutput=True)

    mlin_d = nc.dram_tensor("m_lin", [N], F32)
    rklin_d = nc.dram_tensor("rank_lin", [N], F32)
    mtop_d = nc.dram_tensor("mtop_lin", [144], F32)

    def emit(tc):
        with (
            tc.tile_pool(name="const", bufs=1) as cpool,
            tc.tile_pool(name="big", bufs=1) as bpool,
            tc.tile_pool(name="mask", bufs=4) as mpool,
            tc.tile_pool(name="scr", bufs=2) as spool,
            tc.tile_pool(name="wstream", bufs=8) as wpool,
            tc.tile_pool(name="small", bufs=1) as smpool,
        ):
            # ---- constant loads (batched) ----
            xtr = [cpool.tile([128, N], F32R, name=f"xtr{i}", tag=f"xtr{i}") for i in range(2)]
            for ft in range(2):
                nc.sync.dma_start(xtr[ft][:], xtr_d[ft * 128:(ft + 1) * 128, :])
            wall = cpool.tile([128, 8 * D], F32R, name="wall", tag="wall")
            nc.sync.dma_start(
                wall[:], bass.AP(wall_d, 0, [[D, 128], [128 * D, 8], [1, D]])
            )
            wrb = {nm: [wall[:, (2 * i + ft) * D:(2 * i + ft + 1) * D] for ft in range(2)]
                   for i, nm in enumerate(("q", "k", "v", "a"))}
            xtrw = {}
            misc = cpool.tile([128, 523], F32, name="misc", tag="misc")
            nc.sync.dma_start(misc[:], misc_d[:, :])
            nrow = misc[:, 0:512]
            qiota = misc[:, 512:520]
            ones = misc[:, 520:521]
            badd = [misc[:, 521 + i:522 + i] for i in range(2)]
            onesr = cpool.tile([128, 1], F32R, name="onesr", tag="onesr")
            nc.sync.dma_start(onesr[:], onesr_d[:, :])
            iwrap1 = cpool.tile([16, 64], F32, name="iwrap1", tag="iwrap1")
            nc.sync.dma_start(iwrap1[:], iwrap1_d[:, :])
            ident = cpool.tile([128, 128], BF16, name="ident", tag="ident")
            nc.sync.dma_start(ident[:], id_d[:, :])
            ctb_all = cpool.tile([128, 8 * N], BF16, name="ctb_all", tag="ctb_all")
            nc.sync.dma_start(
                ctb_all[:], bass.AP(ct_d, 0, [[N, 128], [128 * N, 8], [1, N]])
            )
            ctb = [ctb_all[:, nt * N:(nt + 1) * N] for nt in range(8)]
            xb_all = cpool.tile([128, 8 * D], BF16, name="xb_all", tag="xb_all")
            nc.sync.dma_start(
                xb_all[:], bass.AP(xb_d, 0, [[D, 128], [128 * D, 8], [1, D]])
            )
            xb = [xb_all[:, nt * D:(nt + 1) * D] for nt in range(8)]
            xw_all = cpool.tile([128, 2 * (N + D)], BF16, name="xw_all", tag="xw_all")
            nc.sync.dma_start(
                xw_all[:],
                bass.AP(xw_d, 0, [[N + D, 128], [128 * (N + D), 2], [1, N + D]]),
            )
            xtb = [xw_all[:, ft * (N + D):ft * (N + D) + N] for ft in range(2)]
            w2b = [xw_all[:, ft * (N + D) + N:(ft + 1) * (N + D)] for ft in range(2)]

            # ---- phase B: projections ----
            aug0 = smpool.tile([128, D], F32R, tag="aug0")
            aug1 = smpool.tile([12, D], F32R, tag="aug1")
            vmean_row = smpool.tile([1, D], F32R, tag="vmean_row")
            ktT = [bpool.tile([128, N], F32R, name=f"ktT{i}", tag=f"ktT{i}") for i in range(2)]
            qtT = [bpool.tile([128, N], F32R, name=f"qtT{i}", tag=f"qtT{i}") for i in range(2)]
            qn = [bpool.tile([128, D], F16, name=f"qn{i}", tag=f"qn{i}") for i in range(8)]
            psb_dst = [bpool.tile([128, D], F32, name=f"psb{i}", tag=f"psb{i}") for i in range(8)]
            ktT16 = [bpool.tile([128, N], F16, name=f"ktT16_{i}", tag=f"ktT16_{i}") for i in range(2)]
            vn = [bpool.tile([128, D], F32R, name=f"vn{i}", tag=f"vn{i}") for i in range(8)]

            with tc.tile_pool(name="psA", bufs=2, space="PSUM") as psA, \
                 tc.tile_pool(name="psB", bufs=2, space="PSUM") as psB, \
                 tc.tile_pool(name="psVm", bufs=1, space="PSUM") as psVm:
                # K^T / Q^T (fp32), R^T (fp32r): out [e-chunk 128, n]
                for name, wt, xs, dst in (
                    ("k", wrb["k"], xtr, ktT),
                    ("q", wrb["q"], xtr, qtT),
                ):
                    for et in range(2):
                        ps = psA.tile([128, N], F32, tag="psA")
                        for nck in range(2):
                            for ft in range(2):
                                nc.tensor.matmul(
                                    ps[:, nck * 512:(nck + 1) * 512],
                                    wt[ft][:, et * 128:(et + 1) * 128],
                                    xs[ft][:, nck * 512:(nck + 1) * 512],
                                    start=(ft == 0), stop=(ft == 1),
                                )
                        nc.scalar.copy(dst[et][:], ps[:])
                        if name == "k":
                            nc.scalar.copy(ktT16[et][:], ps[:])

                # Q natural + V natural via f32r inputs; vmean lands on
                # psum partition 12 (e12 selector column) for a lane-local evict
                for qt in range(8):
                    for dst, w in ((qn, "q"), (vn, "v")):
                        ps = psB.tile([128, D], F32, tag="psB")
                        for ft in range(2):
                            nc.tensor.matmul(
                                ps[:],
                                xtr[ft][:, qt * 128:(qt + 1) * 128],
                                wrb[w][ft][:],
                                start=(ft == 0), stop=(ft == 1),
                            )
                        nc.scalar.copy(dst[qt][:], ps[:])
                # P = X @ W2 (bf16) for the PE-side sample-sum
                for qt in range(8):
                    psP = psB.tile([128, D], F32, tag="psB")
                    for ft in range(2):
                        nc.tensor.matmul(
                            psP[:], xtb[ft][:, qt * 128:(qt + 1) * 128], w2b[ft][:],
                            start=(ft == 0), stop=(ft == 1),
                        )
                    nc.scalar.copy(psb_dst[qt][:], psP[:])
                vmean_ps = psVm.tile([1, D], F32, tag="vmean")
                for qt in range(8):
                    nc.tensor.matmul(
                        vmean_ps[:], onesr[:], vn[qt][:],
                        start=(qt == 0), stop=(qt == 7),
                    )
                nc.scalar.mul(vmean_row[:], vmean_ps[:], 1.0 / N)

            if stage == 1:
                nc.sync.dma_start(out_d[:, 0:8], ktT[0][0:1, 0:8])
                return
            # ---- phase C: QK + M ----
            maxacc = smpool.tile([128, 8], F32, tag="maxacc")
            sumacc = smpool.tile([128, 8], F32, tag="sumacc")
            with tc.tile_pool(name="psQK", bufs=2, space="PSUM") as psQK:
                for qt in range(8):
                    am = mpool.tile([128, N], BF16, tag="am")
                    nc.sync.dma_start(am[:], am_d[qt * 128:(qt + 1) * 128, :])
                    qk = psQK.tile([128, N], F32, tag="qk")
                    for kc in range(2):
                        for et in range(2):
                            nc.tensor.matmul(
                                qk[:, kc * 512:(kc + 1) * 512],
                                qtT[et][:, qt * 128:(qt + 1) * 128],
                                ktT[et][:, kc * 512:(kc + 1) * 512],
                                start=(et == 0), stop=False,
                            )
                        nc.tensor.matmul(
                            qk[:, kc * 512:(kc + 1) * 512], ident[:],
                            am[:, kc * 512:(kc + 1) * 512],
                            start=False, stop=True,
                        )
                    if stage == 15 and qt == 0:
                        dbg = smpool.tile([1, N], F32, tag="dbg15")
                        nc.scalar.copy(dbg[:], qk[0:1, :])
                        nc.sync.dma_start(out_d[:, :], dbg[0:1, 0:20])
                        return
                    emit.last_msk = qk
                    if stage == 17:
                        continue
                    nc.vector.tensor_reduce(
                        maxacc[:, qt:qt + 1], qk[:], mybir.AxisListType.X, ALU.max
                    )
                    if stage == 18:
                        continue
                    psX = psQK.tile([128, D], F32, tag="psX", bufs=2)
                    for nt in range(8):
                        nc.tensor.matmul(
                            psX[:], ctb[nt][:, qt * 128:(qt + 1) * 128], xb[nt][:],
                            start=(nt == 0), stop=(nt == 7),
                        )
                    scr2 = spool.tile([128, D], F32, tag="scr256")
                    nc.vector.scalar_tensor_tensor(
                        scr2[:], psX[:], 1.0 / N, psb_dst[qt][:], ALU.mult, ALU.mult,
                        accum_out=sumacc[:, qt:qt + 1],
                    )
            if stage in (16, 17, 18):
                if stage == 17:
                    nc.sync.dma_start(out_d[:, :], emit.last_msk[0:1, 0:20])
                elif stage == 18:
                    nc.sync.dma_start(out_d[:, 0:8], maxacc[0:1, :])
                else:
                    nc.sync.dma_start(out_d[:, 0:8], maxacc[0:1, :])
                    nc.sync.dma_start(out_d[:, 8:16], sumacc[0:1, :])
                return
            m_sb = smpool.tile([128, 8], F32, tag="m_sb")
            nc.vector.tensor_sub(m_sb[:], maxacc[:], sumacc[:])

            if stage == 2:
                nc.sync.dma_start(out_d[:, 0:8], m_sb[0:1, :])
                return
            # ---- phase D: rank-based top-140 selection ----
            # rank[q] = #{j : M[j] > M[q]}; select rank < 140 (ties resolve
            # by index order in the compaction, matching jax.lax.top_k).
            nc.sync.dma_start(bass.AP(mlin_d, 0, [[8, 128], [1, 8]]), m_sb[:])
            mbc = smpool.tile([128, N], F32, tag="mbc")
            nc.sync.dma_start(mbc[:], bass.AP(mlin_d, 0, [[0, 128], [1, 1024]]))
            rank = smpool.tile([128, 8], F32, tag="rank")
            for qt in range(8):
                scrap = spool.tile([128, N], F32, tag="scr")
                nc.vector.tensor_scalar(
                    scrap[:], mbc[:], m_sb[:, qt:qt + 1], None, ALU.is_gt,
                    ALU.add, accum_out=rank[:, qt:qt + 1],
                )
            if stage == 3:
                nc.sync.dma_start(out_d[:, 0:8], rank[0:1, :])
                return
            nc.sync.dma_start(bass.AP(rklin_d, 0, [[8, 128], [1, 8]]), rank[:])
            rkw = smpool.tile([16, 64], F32, tag="rkw")
            nc.sync.dma_start(rkw[:], bass.AP(rklin_d, 0, [[64, 16], [1, 64]]))
            selm = smpool.tile([16, 64], F32, tag="selm")
            nc.vector.tensor_scalar(selm[:], rkw[:], 139.5, None, ALU.is_le)
            selv = smpool.tile([16, 64], F32, tag="selv")
            nc.vector.tensor_mul(selv[:], selm[:], iwrap1[:])
            selv2 = smpool.tile([16, 64], F32, tag="selv2")
            nc.vector.tensor_scalar(selv2[:], selv[:], -1.0, None, ALU.add)
            compact = smpool.tile([16, 64], F32, tag="compact")
            nfound = smpool.tile([1, 1], mybir.dt.uint32, tag="nfound")
            nc.gpsimd.sparse_gather(compact[:], selv2[:], num_found=nfound[:])
            nc.scalar.dma_start(
                bass.AP(mtop_d, 0, [[1, 16], [16, 9]]), compact[:, 0:9]
            )
            mtc0 = smpool.tile([128, 1], F32, tag="mtc0")
            nc.scalar.dma_start(mtc0[:], bass.AP(mtop_d, 0, [[1, 128], [1, 1]]))
            mtc1 = smpool.tile([12, 1], F32, tag="mtc1")
            nc.scalar.dma_start(mtc1[:], bass.AP(mtop_d, 128, [[1, 12], [1, 1]]))
            mrow = smpool.tile([128, 144], F32, tag="mrow")
            nc.scalar.dma_start(mrow[:], bass.AP(mtop_d, 0, [[0, 128], [1, 144]]))

            if stage == 4:
                nc.sync.dma_start(out_d[:, 0:9], compact[0:1, 0:9])
                nc.sync.dma_start(out_d[:, 9:10], mtc0[0:1, :])
                nc.sync.dma_start(out_d[:, 10:11], mrow[127:128, 0:1])
                return
            # ---- phase E: Q_red, scores^T, softmax, attn@V ----
            e2 = [smpool.tile([128, U], F16, name=f"e2_{i}", tag=f"e2_{i}") for i in range(8)]
            for qt in range(8):
                nc.vector.tensor_scalar(
                    e2[qt][:], mrow[:, 0:U], qiota[:, qt:qt + 1], None, ALU.is_equal
                )
            qredT = [smpool.tile([128, U], F16, name=f"qredT{i}", tag=f"qredT{i}") for i in range(2)]
            expdT = [smpool.tile([128, U], F32R, name=f"expdT{i}", tag=f"expdT{i}") for i in range(8)]
            with tc.tile_pool(name="psC", bufs=2, space="PSUM") as psC, \
                 tc.tile_pool(name="psD", bufs=2, space="PSUM") as psD, \
                 tc.tile_pool(name="psE", bufs=2, space="PSUM") as psE:
                for ec in range(2):
                    ps = psC.tile([128, U], F32, tag="psC")
                    for qt in range(8):
                        nc.tensor.matmul(
                            ps[:], qn[qt][:, ec * 128:(ec + 1) * 128], e2[qt][:],
                            start=(qt == 0), stop=(qt == 7),
                        )
                    nc.scalar.copy(qredT[ec][:], ps[:])
                for kt in range(8):
                    ps = psC.tile([128, U], F32, tag="psC")
                    for et in range(2):
                        nc.tensor.matmul(
                            ps[:], ktT16[et][:, kt * 128:(kt + 1) * 128], qredT[et][:],
                            start=(et == 0), stop=(et == 1),
                        )
                    nc.scalar.activation(
                        expdT[kt][:], ps[:], ACTF.Exp, scale=1.0 / math.sqrt(D)
                    )
                recip = []
                for sc, (p0, np_) in enumerate(((0, 128), (128, 12))):
                    psd = psD.tile([np_, 1], F32, tag="psD")
                    for kt in range(8):
                        nc.tensor.matmul(
                            psd[:], expdT[kt][:, p0:p0 + np_].bitcast(F32), ones[:],
                            start=(kt == 0), stop=(kt == 7),
                        )
                    rc = smpool.tile([np_, 1], F32, tag=f"recip{sc}")
                    nc.vector.reciprocal(rc[:], psd[:])
                    recip.append(rc)
                for sc, (p0, np_) in enumerate(((0, 128), (128, 12))):
                    ps = psE.tile([np_, D], F32, tag="psE")
                    for kt in range(8):
                        nc.tensor.matmul(
                            ps[:], expdT[kt][:, p0:p0 + np_], vn[kt][:],
                            start=(kt == 0), stop=(kt == 7),
                        )
                    dst = aug0[:] if sc == 0 else aug1[:]
                    nc.vector.tensor_scalar(
                        dst, ps[:], recip[sc][:], None, ALU.mult
                    )

            if stage == 5:
                nc.sync.dma_start(out_d[:, :], aug0[0:1, 0:20].bitcast(F32))
                return
            # ---- phase F: half-width scatter + residual in PSUM + fused dots ----
            H = 512
            er0 = smpool.tile([128, H], F32R, tag="er0")
            nc.vector.tensor_scalar(er0[:], nrow[:, 0:H], mtc0[:], None, ALU.is_equal)
            er1 = smpool.tile([12, H], F32R, tag="er1")
            nc.vector.tensor_scalar(
                er1[:], nrow[0:12, 0:H], mtc1[:], None, ALU.is_equal
            )
            fill_row = smpool.tile([1, H], F32R, tag="fill_row")
            facc = smpool.tile([128, NCLS], F32, tag="facc")
            with tc.tile_pool(name="psF", bufs=2, space="PSUM") as psF, \
                 tc.tile_pool(name="psCt", bufs=1, space="PSUM") as psCt:
                cnt = psCt.tile([1, H], F32, tag="cnt")
                nc.tensor.matmul(cnt[:], onesr[:], er0[:], start=True, stop=False)
                nc.tensor.matmul(cnt[:], onesr[0:12, :], er1[:],
                                 start=False, stop=True)
                nc.scalar.activation(fill_row[:], cnt[:], ACTF.Copy,
                                     bias=1.0, scale=-1.0)
                ctxh = smpool.tile([128, 2 * H], BF16, name="ctxh", tag="ctxh")
                for dtl in range(2):
                    ds = slice(dtl * 128, (dtl + 1) * 128)
                    ps = psF.tile([128, H], F32, tag="psF")
                    for ft in range(2):
                        nc.tensor.matmul(ps[:], wrb["a"][ft][:, ds],
                                         xtr[ft][:, 0:H],
                                         start=(ft == 0), stop=False)
                    nc.tensor.matmul(ps[:], aug0[:, ds], er0[:],
                                     start=False, stop=False)
                    nc.tensor.matmul(ps[:], aug1[:, ds], er1[:],
                                     start=False, stop=False)
                    nc.tensor.matmul(ps[:], vmean_row[0:1, ds], fill_row[0:1, :],
                                     start=False, stop=True)
                    nc.scalar.activation(ctxh[:, dtl * H:(dtl + 1) * H], ps[:],
                                         ACTF.Identity, bias=badd[dtl][:], scale=1.0)
                for cls in range(NCLS):
                    w = wpool.tile([128, 2 * H], BF16, tag="wf")
                    nc.sync.dma_start(w[:], wf_d[cls, :, :])
                    scr = spool.tile([128, 2 * H], BF16, tag="scrf")
                    nc.vector.scalar_tensor_tensor(
                        scr[:], ctxh[:], 1.0, w[:], ALU.mult, ALU.mult,
                        accum_out=facc[:, cls:cls + 1],
                    )
            with tc.tile_pool(name="psO", bufs=1, space="PSUM") as psO:
                o = psO.tile([1, NCLS], F32, tag="o")
                nc.tensor.matmul(o[:], ones[:], facc[:], start=True, stop=True)
                osb = smpool.tile([1, NCLS], F32, tag="osb")
                nc.scalar.copy(osb[:], o[:])
                nc.sync.dma_start(out_d[:, :], osb[:])

    with tile.TileContext(nc) as tc:
        emit(tc)
    nc.compile()
    return nc


_NC_CACHE = {}


def get_nc(stage=6):
    if stage not in _NC_CACHE:
        _NC_CACHE[stage] = build_nc(stage)
    return _NC_CACHE[stage]


def host_prep(inputs):
    """Build per-core input maps from the full problem inputs."""
    x = np.asarray(inputs["input_embedding"], np.float32)        # [B, N, D]
    wq = np.asarray(inputs["Wq"], np.float32)
    wk = np.asarray(inputs["Wk"], np.float32)
    wv = np.asarray(inputs["Wv"], np.float32)
    wa = np.asarray(inputs["Wadd"], np.float32)
    badd = np.asarray(inputs["badd"], np.float32)
    wfin = np.asarray(inputs["Wfin"], np.float32)                # [10, N*D]
    idx = np.asarray(inputs["index_sample"]).astype(np.int64)    # [N, U]

    cnt = np.zeros((N, N), np.float32)
    np.add.at(cnt, (np.arange(N)[:, None], idx), 1.0)

    # The device program always dots ctx[:, 0:512] against its Wfin shard.
    # Core half h=1 gets the n-axis halves swapped on every n-indexed input
    # (the pipeline is equivariant under a joint permutation of X rows,
    # mask rows+cols, and Wfin columns), so "columns 0:512" is its half.
    perms = [np.arange(N), np.concatenate([np.arange(512, N), np.arange(512)])]
    amask_h, ct_h = [], []
    for p in perms:
        cp = cnt[p][:, p]
        amask_h.append(np.where(cp > 0, 0.0, NEG).astype(ml_dtypes.bfloat16))
        ct_h.append(np.ascontiguousarray(cp.T).astype(ml_dtypes.bfloat16))

    # Wfin[c, n*256+d] -> [10, d, n] -> halves -> [20, 128, 512]
    wr = wfin.reshape(NCLS, N, D).transpose(0, 2, 1)             # [10, 256, 1024]
    wr_h = [
        np.ascontiguousarray(
            wr[:, :, h * 512:(h + 1) * 512].reshape(NCLS, 2, 128, 512)
            .transpose(0, 2, 1, 3)                               # [10,128,2,512]
        ).reshape(NCLS, 128, 2 * 512).astype(ml_dtypes.bfloat16)
        for h in range(2)
    ]

    def trunc22(a):
        b = np.ascontiguousarray(a, np.float32)
        return (b.view(np.uint32) & np.uint32(0xFFFFFC00)).view(np.float32)

    _ii = (np.arange(16, dtype=np.float32)[:, None] * 64
           + np.arange(64, dtype=np.float32)[None, :])
    w_all = np.stack([trunc22(w.T).reshape(2, 128, D) for w in (wq, wk, wv, wa)])
    misc = np.zeros((128, 523), np.float32)
    misc[:, 0:512] = np.arange(512, dtype=np.float32)[None, :]
    misc[:, 512:520] = (np.arange(128, dtype=np.float32)[:, None]
                        + 128.0 * np.arange(8, dtype=np.float32)[None, :])
    misc[:, 520] = 1.0
    misc[:, 521] = badd[0:128]
    misc[:, 522] = badd[128:256]
    consts = {
        "w_all_r": np.ascontiguousarray(w_all.reshape(8, 128, D)),
        "ones_r": np.ones((128, 1), np.float32),
        "ident_b": np.eye(128, dtype=np.float32).astype(ml_dtypes.bfloat16),
        "misc": misc,
        "iwrap1": (1.0 + (_ii % 8) * 128 + _ii // 8),
    }

    in_maps = []
    xt_cache = {}
    for c in range(8):
        b, h = c // 2, c % 2
        m = dict(consts)
        if (b, h) not in xt_cache:
            xp = np.ascontiguousarray(x[b][perms[h]])
            xtc = np.ascontiguousarray(xp.T)
            w2 = (wq.T @ wk).astype(np.float32)
            xtw2 = np.concatenate([xtc, w2], axis=1)             # [256, N+D]
            xt_cache[(b, h)] = (trunc22(xtc), xp.astype(ml_dtypes.bfloat16),
                                xtw2.astype(ml_dtypes.bfloat16))
        m["xt_r"], m["x_b"], m["xtw2_b"] = xt_cache[(b, h)]
        m["amask"] = amask_h[h]
        m["ct_b"] = ct_h[h]
        m["wfin_r"] = wr_h[h]
        in_maps.append(m)
    return in_maps


def host_combine(results, inputs):
    bfin = np.asarray(inputs["bfin"], np.float32)
    out = np.zeros((B, NCLS), np.float32)
    for c in range(8):
        b = c // 2
        out[b] += results[c]["out20"].reshape(NCLS)
    return out + bfin[None, :]


def kernel(**inputs):
    nc = get_nc()
    in_maps = host_prep(inputs)
    res = run_bass_kernel_spmd(nc, in_maps, core_ids=list(range(8)))
    return host_combine(res.results, inputs)

